# revision 1
# baseline (speedup 1.0000x reference)
"""Trainium2 Bass kernel for nn_LiteTransformer (sparse_attention).

Sharding (8 cores):
  - position-attention (down & up): by head (core c owns head c)
  - self-attention blocks: core c owns batch c//2, heads c%2*4..+4
  - decoder: token-sharded (2048 tokens per core)
Cross-core exchange: AllGather collectives (bf16, DRAM bounce).
Host preprocessing: grid/concat/transpose of the encoder input, percentile
thresholds + masked distance matrices (pure functions of input m_cross),
weight packing/casting.
"""

import numpy as np
import ml_dtypes

import concourse.bass as bass
import concourse.mybir as mybir
import concourse.tile as tile
from concourse import bacc
from concourse.bass import ds
from concourse.bass_utils import run_bass_kernel_spmd
from concourse.masks import make_identity

BF = mybir.dt.bfloat16
F32 = mybir.dt.float32
AF = mybir.ActivationFunctionType
NPBF = ml_dtypes.bfloat16

B, RES, N, M, H, D, KD, NB = 4, 64, 4096, 1024, 8, 256, 32, 4
BN = B * N
NCORE = 8
INV_SQRT_K = float(1.0 / np.sqrt(np.float32(KD)))
ALL8 = [list(range(NCORE))]
PAIRS = [[0, 1], [2, 3], [4, 5], [6, 7]]

_cache = {}


def _build():
    nc = bacc.Bacc("TRN2", target_bir_lowering=False, debug=False,
                   num_devices=NCORE)

    def din(name, shape, dt=BF):
        return nc.dram_tensor(name, list(shape), dt, kind="ExternalInput").ap()

    enc_inT = din("enc_inT", (4, BN))
    mdown = din("mdown", (N, M))
    mupT = din("mupT", (M, N))
    wen = din("wen", (4, D))
    ben = din("ben", (D, 1), F32)
    wdown = din("wdown", (D, KD))
    ncdown = din("ncdown", (128, 1), F32)
    wup = din("wup", (D, KD))
    ncup = din("ncup", (128, 1), F32)
    qp = din("qp", (NB, D, 128))
    kp = din("kp", (NB, D, 128))
    vp = din("vp", (NB, D, 128))
    w1 = din("w1", (NB, D, D))
    b1 = din("b1", (NB, D, 1), F32)
    w2 = din("w2", (NB, D, D))
    wr = din("wr", (NB, D, D))
    bcomb = din("bcomb", (NB, D, 1), F32)
    wde1 = din("wde1", (D, D))
    bde1 = din("bde1", (D, 1), F32)
    wde2 = din("wde2", (D, 1))
    out_shard = nc.dram_tensor("out_shard", [1, BN // NCORE], F32,
                               kind="ExternalOutput").ap()

    with tile.TileContext(nc) as tc:
        with (
            tc.tile_pool(name="dram", bufs=1, space="DRAM") as dram,
            tc.tile_pool(name="consts", bufs=1) as consts,
            tc.tile_pool(name="small", bufs=6) as small,
            tc.tile_pool(name="pp", bufs=4, space="PSUM") as pp,
            tc.tile_pool(name="pt", bufs=2, space="PSUM") as ppt,
        ):
            ident = consts.tile([128, 128], BF, name="ident", tag="ident")
            make_identity(nc, ident)
            pid = nc.sync.partition_id()

            ag1_in = dram.tile([128, M], BF, name="ag1i", tag="ag1i")
            ag1_out = dram.tile([NCORE * 128, M], BF, name="ag1o", tag="ag1o", addr_space="Shared")
            ag3_in = dram.tile([D, M], BF, name="ag3i", tag="ag3i")
            ag3_out = dram.tile([NCORE * D, M], BF, name="ag3o", tag="ag3o", addr_space="Shared")
            ag4_in = dram.tile([KD, BN], BF, name="ag4i", tag="ag4i")
            ag4_out = dram.tile([NCORE * KD, BN], BF, name="ag4o", tag="ag4o", addr_space="Shared")

            def psum(p, f, dt=F32):
                return pp.tile([p, f], dt, name="pp", tag="pp")

            _lwn = [0]

            def lw(pool, src, p0, p1, f0, f1, dt=BF):
                _lwn[0] += 1
                t = pool.tile([p1 - p0, f1 - f0], dt, name=f"lw{_lwn[0]}", tag=f"lw{_lwn[0]}")
                nc.sync.dma_start(t[:], src[p0:p1, f0:f1])
                return t

            # ---------------- P1: down ----------------
            with tc.tile_pool(name="p1", bufs=3) as p1, \
                 tc.tile_pool(name="p1keep", bufs=1) as p1k:
                wen_sb = lw(consts, wen, 0, 4, 0, D)
                enc_sb = consts.tile([4, BN], BF, name="enc_sb", tag="enc_sb")
                nc.sync.dma_start(enc_sb[:], enc_inT[:, :])
                ben_sb = [lw(consts, ben, t * 128, t * 128 + 128, 0, 1, F32)
                          for t in range(2)]
                wdn_sb = [lw(consts, wdown, t * 128, t * 128 + 128, 0, KD)
                          for t in range(2)]
                ncd_sb = lw(consts, ncdown, 0, 128, 0, 1, F32)

                v_all = [p1k.tile([128, 4 * 33], BF, name=f"va{i}", tag=f"va{i}") for i in range(32)]
                for b in range(B):
                    for ni in range(32):
                        off = b * N + ni * 128
                        enT = []
                        for t in range(2):
                            pe = psum(128, 128)
                            nc.tensor.matmul(
                                pe[:], wen_sb[:, t * 128:(t + 1) * 128],
                                enc_sb[:, off:off + 128])
                            g = p1.tile([128, 128], BF, name="enT", tag="enT")
                            nc.scalar.activation(g[:], pe[:], AF.Gelu,
                                                 bias=ben_sb[t][:])
                            enT.append(g)
                        pv = psum(128, KD)
                        for t in range(2):
                            nc.tensor.matmul(pv[:], enT[t][:], wdn_sb[t][:],
                                             start=(t == 0), stop=(t == 1))
                        nc.vector.tensor_copy(
                            v_all[ni][:, b * 33:b * 33 + KD], pv[:])
                        if b == 0:
                            for bb in range(B):
                                nc.vector.memset(
                                    v_all[ni][:, bb * 33 + 32:bb * 33 + 33],
                                    1.0)

                a_sb = [p1k.tile([128, M], BF, name=f"as{i}", tag=f"as{i}") for i in range(32)]
                for ni in range(32):
                    st = p1.tile([128, M], BF, name="mstage", tag="mstage")
                    nc.sync.dma_start(st[:],
                                      mdown[ni * 128:(ni + 1) * 128, :])
                    nc.scalar.activation(a_sb[ni][:], st[:], AF.Exp,
                                         scale=ncd_sb[:])

                xhT = p1k.tile([128, M], BF, name="xhT", tag="xhT")
                for mi in range(8):
                    px = psum(128, 4 * 33)
                    for ni in range(32):
                        nc.tensor.matmul(
                            px[:], a_sb[ni][:, mi * 128:(mi + 1) * 128],
                            v_all[ni][:], start=(ni == 0), stop=(ni == 31))
                    for b in range(B):
                        rc = small.tile([128, 1], F32, name="rc", tag="rc")
                        nc.vector.reciprocal(
                            rc[:], px[:, b * 33 + 32:b * 33 + 33])
                        gx = small.tile([128, KD], BF, name="gx", tag="gx")
                        nc.scalar.activation(gx[:], px[:, b * 33:b * 33 + KD],
                                             AF.Gelu, scale=rc[:])
                        ptr = ppt.tile([KD, 128], BF, name="tp", tag="tp")
                        nc.tensor.transpose(ptr[:], gx[:], ident[:])
                        nc.vector.tensor_copy(
                            xhT[b * 32:b * 32 + 32,
                                mi * 128:(mi + 1) * 128], ptr[:])
                nc.sync.dma_start(ag1_in[:], xhT[:])
                nc.gpsimd.collective_compute(
                    "AllGather", mybir.AluOpType.bypass, replica_groups=ALL8,
                    ins=[ag1_in.opt()], outs=[ag1_out.opt()])

            # ---------------- P2: blocks ----------------
            b0x32 = (pid // 2) * 32
            with tc.tile_pool(name="p2", bufs=2) as p2, \
                 tc.tile_pool(name="p2e", bufs=1) as p2e:
                xT = [p2e.tile([128, M], BF, name=f"xT{t}", tag=f"xT{t}") for t in range(2)]
                for hh in range(H):
                    nc.sync.dma_start(
                        xT[hh // 4][(hh % 4) * 32:(hh % 4) * 32 + 32, :],
                        ag1_out[ds(hh * 128 + b0x32, 32), :])

                for blk in range(NB):
                    qp_sb = [lw(p2, qp[blk], t * 128, (t + 1) * 128, 0, 128)
                             for t in range(2)]
                    kp_sb = [lw(p2, kp[blk], t * 128, (t + 1) * 128, 0, 128)
                             for t in range(2)]
                    vp_sb = [lw(p2, vp[blk], t * 128, (t + 1) * 128, 0, 128)
                             for t in range(2)]

                    qt = p2e.tile([128, M], BF, name="qt", tag="qt")
                    kt = p2e.tile([128, M], BF, name="kt", tag="kt")
                    for dst, wsb in ((qt, qp_sb), (kt, kp_sb)):
                        for mh in range(2):
                            pq = psum(128, 512)
                            for t in range(2):
                                nc.tensor.matmul(
                                    pq[:], wsb[t][:],
                                    xT[t][:, mh * 512:(mh + 1) * 512],
                                    start=(t == 0), stop=(t == 1))
                            nc.vector.tensor_copy(
                                dst[:, mh * 512:(mh + 1) * 512], pq[:])
                    qh = [p2e.tile([KD, M], BF, name=f"qh{h}", tag=f"qh{h}") for h in range(4)]
                    kh = [p2e.tile([KD, M], BF, name=f"kh{h}", tag=f"kh{h}") for h in range(4)]
                    for h in range(4):
                        nc.vector.tensor_copy(qh[h][:],
                                              qt[h * 32:h * 32 + 32, :])
                        nc.vector.tensor_copy(kh[h][:],
                                              kt[h * 32:h * 32 + 32, :])

                    vh = [p2e.tile([128, 4 * 33], BF, name=f"vh{ni}", tag=f"vh{ni}")
                          for ni in range(8)]
                    for ni in range(8):
                        pvv = psum(128, 128)
                        for t in range(2):
                            nc.tensor.matmul(
                                pvv[:],
                                xT[t][:, ni * 128:(ni + 1) * 128],
                                vp_sb[t][:], start=(t == 0), stop=(t == 1))
                        for h in range(4):
                            nc.vector.tensor_copy(
                                vh[ni][:, h * 33:h * 33 + KD],
                                pvv[:, h * 32:h * 32 + 32])
                            nc.vector.memset(
                                vh[ni][:, h * 33 + 32:h * 33 + 33], 1.0)

                    es = [[p2e.tile([128, M], BF, name=f"es{h}_{ni}", tag=f"es{h}_{ni}")
                           for ni in range(8)] for h in range(4)]
                    for h in range(4):
                        for ni in range(8):
                            for mh in range(2):
                                psc = psum(128, 512)
                                nc.tensor.matmul(
                                    psc[:],
                                    kh[h][:, ni * 128:(ni + 1) * 128],
                                    qh[h][:, mh * 512:(mh + 1) * 512])
                                nc.scalar.activation(
                                    es[h][ni][:, mh * 512:(mh + 1) * 512],
                                    psc[:], AF.Exp, scale=INV_SQRT_K)

                    paT = p2e.tile([128, M], BF, name="paT", tag="paT")
                    for mi in range(8):
                        for h in range(4):
                            pa = psum(128, 33)
                            for ni in range(8):
                                nc.tensor.matmul(
                                    pa[:],
                                    es[h][ni][:, mi * 128:(mi + 1) * 128],
                                    vh[ni][:, h * 33:h * 33 + 33],
                                    start=(ni == 0), stop=(ni == 7))
                            rc = small.tile([128, 1], F32, name="rc", tag="rc")
                            nc.vector.reciprocal(rc[:], pa[:, 32:33])
                            gx = small.tile([128, KD], BF, name="gx", tag="gx")
                            nc.scalar.activation(gx[:], pa[:, 0:KD], AF.Gelu,
                                                 scale=rc[:])
                            ptr = ppt.tile([KD, 128], BF, name="tp", tag="tp")
                            nc.tensor.transpose(ptr[:], gx[:], ident[:])
                            nc.vector.tensor_copy(
                                paT[h * 32:h * 32 + 32,
                                    mi * 128:(mi + 1) * 128], ptr[:])

                    ag2_in = dram.tile([128, M], BF, name="ag2i", tag="ag2i")
                    ag2_out = dram.tile([D, M], BF, name="ag2o", tag="ag2o")
                    nc.sync.dma_start(ag2_in[:], paT[:])
                    nc.gpsimd.collective_compute(
                        "AllGather", mybir.AluOpType.bypass,
                        replica_groups=PAIRS,
                        ins=[ag2_in.opt()], outs=[ag2_out.opt()])
                    paF = [p2e.tile([128, M], BF, name=f"paF{t}", tag=f"paF{t}")
                           for t in range(2)]
                    for t in range(2):
                        nc.sync.dma_start(
                            paF[t][:], ag2_out[t * 128:(t + 1) * 128, :])

                    w1_sb = [[lw(p2, w1[blk], i * 128, (i + 1) * 128,
                                 o * 128, (o + 1) * 128) for o in range(2)]
                             for i in range(2)]
                    b1_sb = [lw(p2, b1[blk], t * 128, (t + 1) * 128, 0, 1,
                                F32) for t in range(2)]
                    h1 = [p2e.tile([128, M], BF, name=f"h1{t}", tag=f"h1{t}")
                          for t in range(2)]
                    for o in range(2):
                        for mh in range(2):
                            ph = psum(128, 512)
                            for i in range(2):
                                nc.tensor.matmul(
                                    ph[:], w1_sb[i][o][:],
                                    paF[i][:, mh * 512:(mh + 1) * 512],
                                    start=(i == 0), stop=(i == 1))
                            nc.scalar.activation(
                                h1[o][:, mh * 512:(mh + 1) * 512], ph[:],
                                AF.Gelu, bias=b1_sb[o][:])

                    w2_sb = [[lw(p2, w2[blk], i * 128, (i + 1) * 128,
                                 o * 128, (o + 1) * 128) for o in range(2)]
                             for i in range(2)]
                    wr_sb = [[lw(p2, wr[blk], i * 128, (i + 1) * 128,
                                 o * 128, (o + 1) * 128) for o in range(2)]
                             for i in range(2)]
                    bc_sb = [lw(p2, bcomb[blk], t * 128, (t + 1) * 128, 0, 1,
                                F32) for t in range(2)]
                    xn = [p2e.tile([128, M], BF, name=f"xn{t}", tag=f"xn{t}")
                          for t in range(2)]
                    for o in range(2):
                        for mh in range(2):
                            po = psum(128, 512)
                            nc.tensor.matmul(
                                po[:], w2_sb[0][o][:],
                                h1[0][:, mh * 512:(mh + 1) * 512],
                                start=True, stop=False)
                            nc.tensor.matmul(
                                po[:], w2_sb[1][o][:],
                                h1[1][:, mh * 512:(mh + 1) * 512],
                                start=False, stop=False)
                            nc.tensor.matmul(
                                po[:], wr_sb[0][o][:],
                                xT[0][:, mh * 512:(mh + 1) * 512],
                                start=False, stop=False)
                            nc.tensor.matmul(
                                po[:], wr_sb[1][o][:],
                                xT[1][:, mh * 512:(mh + 1) * 512],
                                start=False, stop=True)
                            nc.scalar.activation(
                                xn[o][:, mh * 512:(mh + 1) * 512], po[:],
                                AF.Gelu, bias=bc_sb[o][:])
                    xT = xn

                for t in range(2):
                    nc.sync.dma_start(ag3_in[t * 128:(t + 1) * 128, :],
                                      xT[t][:])
                nc.gpsimd.collective_compute(
                    "AllGather", mybir.AluOpType.bypass, replica_groups=ALL8,
                    ins=[ag3_in.opt()], outs=[ag3_out.opt()])

            # ---------------- P4: up ----------------
            with tc.tile_pool(name="p4", bufs=3) as p4, \
                 tc.tile_pool(name="p4keep", bufs=1) as p4k:
                wup_l = [lw(consts, wup, t * 128, (t + 1) * 128, 0, KD)
                         for t in range(2)]
                ncu_sb = lw(consts, ncup, 0, 128, 0, 1, F32)
                vu = [p4k.tile([128, 4 * 33], BF, name=f"vu{i}", tag=f"vu{i}") for i in range(8)]
                for b in range(B):
                    xb = [p4.tile([128, M], BF, name=f"xb{t}", tag=f"xb{t}")
                          for t in range(2)]
                    for t in range(2):
                        nc.sync.dma_start(
                            xb[t][:],
                            ag3_out[2 * b * D + t * 128:
                                    2 * b * D + (t + 1) * 128, :])
                    for mi in range(8):
                        pv = psum(128, KD)
                        for t in range(2):
                            nc.tensor.matmul(
                                pv[:], xb[t][:, mi * 128:(mi + 1) * 128],
                                wup_l[t][:], start=(t == 0), stop=(t == 1))
                        nc.vector.tensor_copy(
                            vu[mi][:, b * 33:b * 33 + KD], pv[:])
                        if b == 0:
                            for bb in range(B):
                                nc.vector.memset(
                                    vu[mi][:, bb * 33 + 32:bb * 33 + 33], 1.0)

                eu = [p4k.tile([128, N], BF, name=f"eu{i}", tag=f"eu{i}") for i in range(8)]
                for ki in range(8):
                    st = p4.tile([128, N], BF, name="ustage", tag="ustage")
                    nc.sync.dma_start(st[:], mupT[ki * 128:(ki + 1) * 128, :])
                    nc.scalar.activation(eu[ki][:], st[:], AF.Exp,
                                         scale=ncu_sb[:])

                deT = p4k.tile([KD, BN], BF, name="deT", tag="deT")
                for qi in range(32):
                    pd = psum(128, 4 * 33)
                    for ki in range(8):
                        nc.tensor.matmul(
                            pd[:], eu[ki][:, qi * 128:(qi + 1) * 128],
                            vu[ki][:], start=(ki == 0), stop=(ki == 7))
                    for b in range(B):
                        rc = small.tile([128, 1], F32, name="rc", tag="rc")
                        nc.vector.reciprocal(
                            rc[:], pd[:, b * 33 + 32:b * 33 + 33])
                        gx = small.tile([128, KD], BF, name="gx", tag="gx")
                        nc.scalar.activation(gx[:], pd[:, b * 33:b * 33 + KD],
                                             AF.Gelu, scale=rc[:])
                        ptr = ppt.tile([KD, 128], BF, name="tp", tag="tp")
                        nc.tensor.transpose(ptr[:], gx[:], ident[:])
                        nc.vector.tensor_copy(
                            deT[:, b * N + qi * 128:b * N + (qi + 1) * 128],
                            ptr[:])
                nc.sync.dma_start(ag4_in[:], deT[:])
                nc.gpsimd.collective_compute(
                    "AllGather", mybir.AluOpType.bypass, replica_groups=ALL8,
                    ins=[ag4_in.opt()], outs=[ag4_out.opt()])

            # ---------------- P5: decoder on token shard ----------------
            TS = BN // NCORE  # 2048
            toff = pid * TS
            with tc.tile_pool(name="p5", bufs=2) as p5:
                dea = [p5.tile([128, TS], BF, name=f"dea{t}", tag=f"dea{t}")
                       for t in range(2)]
                for t in range(2):
                    nc.sync.dma_start(
                        dea[t][:],
                        ag4_out[t * 128:(t + 1) * 128, ds(toff, TS)])
                wd1 = [[lw(p5, wde1, i * 128, (i + 1) * 128, o * 128,
                           (o + 1) * 128) for o in range(2)]
                       for i in range(2)]
                bd1 = [lw(p5, bde1, t * 128, (t + 1) * 128, 0, 1, F32)
                       for t in range(2)]
                wd2 = [lw(p5, wde2, t * 128, (t + 1) * 128, 0, 1)
                       for t in range(2)]
                g = [p5.tile([128, TS], BF, name=f"g{t}", tag=f"g{t}") for t in range(2)]
                for o in range(2):
                    for th in range(4):
                        pg = psum(128, 512)
                        for i in range(2):
                            nc.tensor.matmul(
                                pg[:], wd1[i][o][:],
                                dea[i][:, th * 512:(th + 1) * 512],
                                start=(i == 0), stop=(i == 1))
                        nc.scalar.activation(
                            g[o][:, th * 512:(th + 1) * 512], pg[:],
                            AF.Gelu, bias=bd1[o][:])
                osb = p5.tile([1, TS], F32, name="osb", tag="osb")
                for th in range(4):
                    p2o = psum(1, 512)
                    for i in range(2):
                        nc.tensor.matmul(
                            p2o[:], wd2[i][:],
                            g[i][:, th * 512:(th + 1) * 512],
                            start=(i == 0), stop=(i == 1))
                    nc.vector.tensor_copy(
                        osb[:, th * 512:(th + 1) * 512], p2o[:])
                nc.sync.dma_start(out_shard[:, :], osb[:])

    nc.compile()
    return nc


def _prep_inputs(inputs, m_cross, W_en, b_en, r_down, w_down, q_pa, k_pa,
                 v_pa, W1_mlp, b1_mlp, W2_mlp, b2_mlp, W_res, b_res, r_up,
                 w_up, W_de1, b_de1, W_de2, b_de2, y_mean, y_std):
    f32 = np.float32
    mc = np.asarray(m_cross, f32)
    # encoder input (B, N, 3) -> (4, BN) transposed, row 3 = 0
    gx = np.linspace(0.0, 1.0, RES + 1, dtype=f32)[:-1]
    gxx = np.broadcast_to(gx[:, None], (RES, RES))
    gyy = np.broadcast_to(gx[None, :], (RES, RES))
    enc = np.empty((B, N, 3), f32)
    enc[:, :, 0] = gxx.reshape(-1)[None, :]
    enc[:, :, 1] = gyy.reshape(-1)[None, :]
    enc[:, :, 2] = np.asarray(inputs, f32).reshape(B, N)
    enc_inT = np.zeros((4, BN), NPBF)
    enc_inT[:3, :] = enc.reshape(BN, 3).T.astype(NPBF)

    c_down = np.tan(0.25 * np.pi * (1.0 + np.sin(np.asarray(r_down, f32)
                                                 .reshape(H)))).astype(f32)
    c_up = np.tan(0.25 * np.pi * (1.0 + np.sin(np.asarray(r_up, f32)
                                               .reshape(H)))).astype(f32)
    kd_ = int(0.30 * (N - 1))          # 1228
    t_down = np.partition(mc, kd_, axis=0)[kd_, :]            # (M,)
    ku_ = int(0.30 * (M - 1))          # 306
    t_up = np.partition(mc, ku_, axis=1)[:, ku_]              # (N,)
    bigd = f32(max(1e4, 400.0 / max(float(c_down.min()), 1e-6)))
    bigu = f32(max(1e4, 400.0 / max(float(c_up.min()), 1e-6)))
    mdown = mc + bigd * (mc > t_down[None, :])                # (N, M)
    mupT = (mc + bigu * (mc > t_up[:, None])).T.copy()        # (M, N)

    wen4 = np.zeros((4, D), NPBF)
    wen4[:3, :] = np.asarray(W_en, f32).astype(NPBF)
    col = lambda a: np.asarray(a, f32).reshape(D, 1)

    w1f = np.asarray(W1_mlp, f32)
    w2f = np.asarray(W2_mlp, f32)
    wrf = np.asarray(W_res, f32)
    bc = (np.asarray(b2_mlp, f32) + np.asarray(b_res, f32))   # (NB, D)
    ystd = float(np.asarray(y_std, f32))
    ymean = float(np.asarray(y_mean, f32))
    wde2f = (np.asarray(W_de2, f32) * ystd).astype(NPBF)      # (D, 1)
    bde2f = float(np.asarray(b_de2, f32).reshape(-1)[0] * ystd + ymean)

    in_maps = []
    for c in range(NCORE):
        h = c
        b0 = c // 2
        hs = [4 * (c % 2) + j for j in range(4)]
        qpc = np.concatenate([np.asarray(q_pa, f32)[:, hh] for hh in hs],
                             axis=2).astype(NPBF)             # (NB, D, 128)
        kpc = np.concatenate([np.asarray(k_pa, f32)[:, hh] for hh in hs],
                             axis=2).astype(NPBF)
        vpc = np.concatenate([np.asarray(v_pa, f32)[:, hh] for hh in hs],
                             axis=2).astype(NPBF)
        im = {
            "enc_inT": enc_inT,
            "mdown": mdown.astype(NPBF),
            "mupT": mupT.astype(NPBF),
            "wen": wen4,
            "ben": col(b_en),
            "wdown": np.asarray(w_down, f32)[h].astype(NPBF),
            "ncdown": np.full((128, 1), -c_down[h], f32),
            "wup": np.asarray(w_up, f32)[h].astype(NPBF),
            "ncup": np.full((128, 1), -c_up[h], f32),
            "qp": qpc, "kp": kpc, "vp": vpc,
            "w1": w1f.astype(NPBF),
            "b1": np.asarray(b1_mlp, f32).reshape(NB, D, 1),
            "w2": w2f.astype(NPBF),
            "wr": wrf.astype(NPBF),
            "bcomb": bc.reshape(NB, D, 1),
            "wde1": np.asarray(W_de1, f32).astype(NPBF),
            "bde1": col(b_de1),
            "wde2": wde2f,
        }
        _ = b0
        in_maps.append(im)
    return in_maps, bde2f


def kernel(**inputs):
    if "nc" not in _cache:
        _cache["nc"] = _build()
    nc = _cache["nc"]
    in_maps, bde2f = _prep_inputs(**inputs)
    res = run_bass_kernel_spmd(nc, in_maps, core_ids=list(range(NCORE)))
    shards = [res.results[c]["out_shard"].reshape(-1) + np.float32(bde2f)
              for c in range(NCORE)]
    out = np.concatenate(shards).astype(np.float32)
    return out.reshape(B, RES, RES, 1)



# revision 9
# speedup vs baseline: 4.1911x; 4.1911x over previous
"""Trainium2 Bass kernel for nn_LiteTransformer (sparse_attention).

Sharding (8 cores):
  - position-attention (down & up): by head (core c owns head c)
  - self-attention blocks: core c owns batch c//2, heads c%2*4..+4
  - decoder: token-sharded (2048 tokens per core)

Host->device transfer is the wall-clock bottleneck (axon tunnel ~120MB/s,
~0.2s per tensor latency), so inputs are packed into THREE small tensors
per core (~1.5MB total) instead of replicating the 16MB masked distance
matrices everywhere:
  - blob  (525,1024) bf16: m_cross row-shard + encoder-input slice +
    masked-percentile thresholds; AllGathered on device.
  - sheet (224,1024) bf16: 1/8 shard of all weights; AllGathered on device.
  - blobS (128,36)  f32 : per-head exp scales + biases (replicated).
The masked distance matrices exp(-c*(mc + big*(mc>thr))) are computed
on-device as exp(-c*mc) * (mc <= thr); thresholds are host-refined so the
bf16 comparison reproduces the exact f32 percentile mask.
"""

import numpy as np
import ml_dtypes

import concourse.bass as bass
import concourse.mybir as mybir
import concourse.tile as tile
from concourse import bacc
from concourse.bass import ds
from concourse.bass_utils import run_bass_kernel_spmd
from concourse.masks import make_identity

BF = mybir.dt.bfloat16
F32 = mybir.dt.float32
AF = mybir.ActivationFunctionType
OP = mybir.AluOpType
NPBF = ml_dtypes.bfloat16

B, RES, N, M, H, D, KD, NB = 4, 64, 4096, 1024, 8, 256, 32, 4
BN = B * N
NCORE = 8
INV_SQRT_K = float(1.0 / np.sqrt(np.float32(KD)))
ALL8 = [list(range(NCORE))]
PAIRS = [[0, 1], [2, 3], [4, 5], [6, 7]]

# blob layout (per-core rows, width 1024 bf16)
BLOB_ROWS = 525          # 512 mc + 8 enc + 1 t_down + 4 t_up
R_ENC = 512
R_TDOWN = 520
R_TUP = 521
# sheet layout (global rows, width 1024 bf16)
SHEET_ROWS = 1792        # 224 per core
SH_W1, SH_W2, SH_WR = 0, 256, 512
SH_QP, SH_KP, SH_VP = 768, 1024, 1280
SH_MISC = 1536           # cols 0:256 wde1 | 256:512 wdown | 512:768 wup | 768: misc2
# misc2: rows SH_MISC..+4 cols 768:1024 = wen; wde2 halves at cols 770,771 rows +8..+136
# blobS cols
SC_NCD, SC_NCU, SC_BEN, SC_B1, SC_BC, SC_BD1, SCOLS = 0, 8, 16, 18, 26, 34, 36

_cache = {}


def _build():
    nc = bacc.Bacc("TRN2", target_bir_lowering=False, debug=False,
                   num_devices=NCORE)

    blob = nc.dram_tensor("blob", [BLOB_ROWS, 1024], BF,
                          kind="ExternalInput").ap()
    sheet = nc.dram_tensor("sheet", [SHEET_ROWS // NCORE, 1024], BF,
                           kind="ExternalInput").ap()
    blobS = nc.dram_tensor("blobS", [128, SCOLS], F32,
                           kind="ExternalInput").ap()
    out_shard = nc.dram_tensor("out_shard", [1, BN // NCORE], F32,
                               kind="ExternalOutput").ap()

    with tile.TileContext(nc) as tc:
        with (
            tc.tile_pool(name="dram", bufs=1, space="DRAM") as dram,
            tc.tile_pool(name="consts", bufs=1) as consts,
            tc.tile_pool(name="small", bufs=6) as small,
            tc.tile_pool(name="pp", bufs=4, space="PSUM") as pp,
            tc.tile_pool(name="pt", bufs=2, space="PSUM") as ppt,
        ):
            ident = consts.tile([128, 128], BF, name="ident", tag="ident")
            make_identity(nc, ident)
            pid = nc.sync.partition_id()

            # ---- gather the packed inputs across cores ----
            # (collectives cannot read IO tensors; bounce through DRAM scratch)
            blob_in = dram.tile([BLOB_ROWS, 1024], BF, name="blobi",
                                tag="blobi")
            nc.sync.dma_start(blob_in[:, :], blob[:, :])
            blob_out = dram.tile([NCORE * BLOB_ROWS, 1024], BF, name="blobo",
                                 tag="blobo", addr_space="Shared")
            nc.gpsimd.collective_compute(
                "AllGather", OP.bypass, replica_groups=ALL8,
                ins=[blob_in.opt()], outs=[blob_out.opt()])
            sheet_in = dram.tile([SHEET_ROWS // NCORE, 1024], BF,
                                 name="sheeti", tag="sheeti")
            nc.sync.dma_start(sheet_in[:, :], sheet[:, :])
            sheet_out = dram.tile([SHEET_ROWS, 1024], BF, name="sheeto",
                                  tag="sheeto", addr_space="Shared")
            nc.gpsimd.collective_compute(
                "AllGather", OP.bypass, replica_groups=ALL8,
                ins=[sheet_in.opt()], outs=[sheet_out.opt()])

            # clean (4096, 1024) m_cross scratch out of the gathered chunks
            mc_dram = dram.tile([N, 1024], BF, name="mcd", tag="mcd")
            for c in range(NCORE):
                nc.sync.dma_start(
                    mc_dram[c * 512:(c + 1) * 512, :],
                    blob_out[c * BLOB_ROWS:c * BLOB_ROWS + 512, :])

            blobS_sb = consts.tile([128, SCOLS], F32, name="bS", tag="bS")
            nc.sync.dma_start(blobS_sb[:], blobS[:, :])
            ncd_sb = consts.tile([128, 1], F32, name="ncd", tag="ncd")
            nc.sync.dma_start(ncd_sb[:], blobS[0:128, ds(SC_NCD + pid, 1)])
            ncu_sb = consts.tile([128, 1], F32, name="ncu", tag="ncu")
            nc.sync.dma_start(ncu_sb[:], blobS[0:128, ds(SC_NCU + pid, 1)])

            # thresholds broadcast to 128 partitions (ones ⊗ row via matmul)
            ones_sb = consts.tile([1, 128], BF, name="ones", tag="ones")
            nc.vector.memset(ones_sb[:], 1.0)
            td_row = consts.tile([1, 1024], BF, name="tdr", tag="tdr")
            nc.sync.dma_start(td_row[:], blob_out[R_TDOWN:R_TDOWN + 1, :])
            thrD = consts.tile([128, 1024], BF, name="thrD", tag="thrD")
            for hf in range(2):
                pb = pp.tile([128, 512], F32, name="pp", tag="pp")
                nc.tensor.matmul(pb[:], ones_sb[:],
                                 td_row[:, hf * 512:(hf + 1) * 512])
                nc.vector.tensor_copy(thrD[:, hf * 512:(hf + 1) * 512], pb[:])

            wen_sb = consts.tile([4, 256], BF, name="wen", tag="wen")
            nc.sync.dma_start(wen_sb[:], sheet_out[SH_MISC:SH_MISC + 4, 768:1024])
            wdn_sb = []
            wup_sb = []
            for t in range(2):
                w = consts.tile([128, KD], BF, name=f"wdn{t}", tag=f"wdn{t}")
                nc.sync.dma_start(
                    w[:], sheet_out[SH_MISC + t * 128:SH_MISC + (t + 1) * 128,
                                    ds(256 + pid * KD, KD)])
                wdn_sb.append(w)
                w = consts.tile([128, KD], BF, name=f"wupt{t}", tag=f"wupt{t}")
                nc.sync.dma_start(
                    w[:], sheet_out[SH_MISC + t * 128:SH_MISC + (t + 1) * 128,
                                    ds(512 + pid * KD, KD)])
                wup_sb.append(w)
            ben_sb = [blobS_sb[:, SC_BEN + t:SC_BEN + t + 1] for t in range(2)]

            ag1_in = dram.tile([128, M], BF, name="ag1i", tag="ag1i")
            ag1_out = dram.tile([NCORE * 128, M], BF, name="ag1o", tag="ag1o",
                                addr_space="Shared")
            ag3_in = dram.tile([D, M], BF, name="ag3i", tag="ag3i")
            ag3_out = dram.tile([NCORE * D, M], BF, name="ag3o", tag="ag3o",
                                addr_space="Shared")
            ag4_in = dram.tile([KD, BN], BF, name="ag4i", tag="ag4i")
            ag4_out = dram.tile([NCORE * KD, BN], BF, name="ag4o", tag="ag4o",
                                addr_space="Shared")

            def psum(p, f, dt=F32):
                return pp.tile([p, f], dt, name="pp", tag="pp")

            _lwn = [0]

            def lw(pool, p0, p1, f0, f1, dt=BF):
                # load a (p1-p0, f1-f0) tile from sheet_out
                _lwn[0] += 1
                t = pool.tile([p1 - p0, f1 - f0], dt, name=f"lw{_lwn[0]}",
                              tag=f"lw{_lwn[0]}")
                nc.sync.dma_start(t[:], sheet_out[p0:p1, f0:f1])
                return t

            # ---------------- P1: down (head pid over full N) ----------------
            with tc.tile_pool(name="p1", bufs=3) as p1, \
                 tc.tile_pool(name="p1keep", bufs=1) as p1k:
                # encoder input (4, BN) from the 16 gathered groups
                enc_sb = p1k.tile([4, BN], BF, name="enc_sb", tag="enc_sb")
                for g in range(16):
                    src = (g // 2) * BLOB_ROWS + R_ENC + (g % 2) * 4
                    nc.sync.dma_start(enc_sb[:, g * 1024:(g + 1) * 1024],
                                      blob_out[src:src + 4, :])
                v_all = [p1k.tile([128, 4 * 33], BF, name=f"va{i}", tag=f"va{i}")
                         for i in range(32)]
                for b in range(B):
                    for ni in range(32):
                        off = b * N + ni * 128
                        enT = []
                        for t in range(2):
                            pe = psum(128, 128)
                            nc.tensor.matmul(
                                pe[:], wen_sb[:, t * 128:(t + 1) * 128],
                                enc_sb[:, off:off + 128])
                            g = p1.tile([128, 128], BF, name="enT", tag="enT")
                            nc.scalar.activation(g[:], pe[:], AF.Gelu,
                                                 bias=ben_sb[t])
                            enT.append(g)
                        pv = psum(128, KD)
                        for t in range(2):
                            nc.tensor.matmul(pv[:], enT[t][:], wdn_sb[t][:],
                                             start=(t == 0), stop=(t == 1))
                        nc.vector.tensor_copy(
                            v_all[ni][:, b * 33:b * 33 + KD], pv[:])
                        if b == 0:
                            for bb in range(B):
                                nc.vector.memset(
                                    v_all[ni][:, bb * 33 + 32:bb * 33 + 33],
                                    1.0)

                # a_sb[ni] = exp(-c_h * mc) * (mc <= thr)
                a_sb = [p1k.tile([128, M], BF, name=f"as{i}", tag=f"as{i}")
                        for i in range(32)]
                for ni in range(32):
                    mct = p1.tile([128, M], BF, name="mct", tag="mct")
                    nc.sync.dma_start(mct[:],
                                      mc_dram[ni * 128:(ni + 1) * 128, :])
                    msk = p1.tile([128, M], BF, name="msk", tag="msk")
                    nc.vector.tensor_tensor(msk[:], mct[:], thrD[:], OP.is_le)
                    nc.scalar.activation(a_sb[ni][:], mct[:], AF.Exp,
                                         scale=ncd_sb[:])
                    nc.vector.tensor_tensor(a_sb[ni][:], a_sb[ni][:], msk[:],
                                            OP.mult)

                xhT = p1k.tile([128, M], BF, name="xhT", tag="xhT")
                for mi in range(8):
                    px = psum(128, 4 * 33)
                    for ni in range(32):
                        nc.tensor.matmul(
                            px[:], a_sb[ni][:, mi * 128:(mi + 1) * 128],
                            v_all[ni][:], start=(ni == 0), stop=(ni == 31))
                    for b in range(B):
                        rc = small.tile([128, 1], F32, name="rc", tag="rc")
                        nc.vector.reciprocal(
                            rc[:], px[:, b * 33 + 32:b * 33 + 33])
                        gx = small.tile([128, KD], BF, name="gx", tag="gx")
                        nc.scalar.activation(gx[:], px[:, b * 33:b * 33 + KD],
                                             AF.Gelu, scale=rc[:])
                        ptr = ppt.tile([KD, 128], BF, name="tp", tag="tp")
                        nc.tensor.transpose(ptr[:], gx[:], ident[:])
                        nc.vector.tensor_copy(
                            xhT[b * 32:b * 32 + 32,
                                mi * 128:(mi + 1) * 128], ptr[:])
                nc.sync.dma_start(ag1_in[:], xhT[:])
                nc.gpsimd.collective_compute(
                    "AllGather", OP.bypass, replica_groups=ALL8,
                    ins=[ag1_in.opt()], outs=[ag1_out.opt()])

            # ---------------- P2: blocks ----------------
            b0x32 = (pid // 2) * 32
            # static range clamp only — the emitted runtime-assert sequencer
            # instruction faults the exec unit on this runtime, so skip it
            hcol = nc.s_assert_within((pid - (pid // 2) * 2) * 128, 0, 128,
                                      skip_runtime_assert=True)
            with tc.tile_pool(name="p2", bufs=2) as p2, \
                 tc.tile_pool(name="p2e", bufs=1) as p2e:
                xT = [p2e.tile([128, M], BF, name=f"xT{t}", tag=f"xT{t}")
                      for t in range(2)]
                for hh in range(H):
                    nc.sync.dma_start(
                        xT[hh // 4][(hh % 4) * 32:(hh % 4) * 32 + 32, :],
                        ag1_out[ds(hh * 128 + b0x32, 32), :])

                for blk in range(NB):
                    qp_sb, kp_sb, vp_sb = [], [], []
                    for t in range(2):
                        for dst, base in ((qp_sb, SH_QP), (kp_sb, SH_KP),
                                          (vp_sb, SH_VP)):
                            _lwn[0] += 1
                            w = p2.tile([128, 128], BF, name=f"lw{_lwn[0]}",
                                        tag=f"lw{_lwn[0]}")
                            nc.sync.dma_start(
                                w[:], sheet_out[base + t * 128:
                                                base + (t + 1) * 128,
                                                ds(blk * 256 + hcol, 128)])
                            dst.append(w)

                    qt = p2e.tile([128, M], BF, name="qt", tag="qt")
                    kt = p2e.tile([128, M], BF, name="kt", tag="kt")
                    for dst, wsb in ((qt, qp_sb), (kt, kp_sb)):
                        for mh in range(2):
                            pq = psum(128, 512)
                            for t in range(2):
                                nc.tensor.matmul(
                                    pq[:], wsb[t][:],
                                    xT[t][:, mh * 512:(mh + 1) * 512],
                                    start=(t == 0), stop=(t == 1))
                            nc.vector.tensor_copy(
                                dst[:, mh * 512:(mh + 1) * 512], pq[:])
                    qh = [p2e.tile([KD, M], BF, name=f"qh{h}", tag=f"qh{h}")
                          for h in range(4)]
                    kh = [p2e.tile([KD, M], BF, name=f"kh{h}", tag=f"kh{h}")
                          for h in range(4)]
                    for h in range(4):
                        nc.vector.tensor_copy(qh[h][:],
                                              qt[h * 32:h * 32 + 32, :])
                        nc.vector.tensor_copy(kh[h][:],
                                              kt[h * 32:h * 32 + 32, :])

                    vh = [p2e.tile([128, 4 * 33], BF, name=f"vh{ni}",
                                   tag=f"vh{ni}") for ni in range(8)]
                    for ni in range(8):
                        pvv = psum(128, 128)
                        for t in range(2):
                            nc.tensor.matmul(
                                pvv[:],
                                xT[t][:, ni * 128:(ni + 1) * 128],
                                vp_sb[t][:], start=(t == 0), stop=(t == 1))
                        for h in range(4):
                            nc.vector.tensor_copy(
                                vh[ni][:, h * 33:h * 33 + KD],
                                pvv[:, h * 32:h * 32 + 32])
                            nc.vector.memset(
                                vh[ni][:, h * 33 + 32:h * 33 + 33], 1.0)

                    es = [[p2e.tile([128, M], BF, name=f"es{h}_{ni}",
                                    tag=f"es{h}_{ni}")
                           for ni in range(8)] for h in range(4)]
                    for h in range(4):
                        for ni in range(8):
                            for mh in range(2):
                                psc = psum(128, 512)
                                nc.tensor.matmul(
                                    psc[:],
                                    kh[h][:, ni * 128:(ni + 1) * 128],
                                    qh[h][:, mh * 512:(mh + 1) * 512])
                                nc.scalar.activation(
                                    es[h][ni][:, mh * 512:(mh + 1) * 512],
                                    psc[:], AF.Exp, scale=INV_SQRT_K)

                    paT = p2e.tile([128, M], BF, name="paT", tag="paT")
                    for mi in range(8):
                        for h in range(4):
                            pa = psum(128, 33)
                            for ni in range(8):
                                nc.tensor.matmul(
                                    pa[:],
                                    es[h][ni][:, mi * 128:(mi + 1) * 128],
                                    vh[ni][:, h * 33:h * 33 + 33],
                                    start=(ni == 0), stop=(ni == 7))
                            rc = small.tile([128, 1], F32, name="rc", tag="rc")
                            nc.vector.reciprocal(rc[:], pa[:, 32:33])
                            gx = small.tile([128, KD], BF, name="gx", tag="gx")
                            nc.scalar.activation(gx[:], pa[:, 0:KD], AF.Gelu,
                                                 scale=rc[:])
                            ptr = ppt.tile([KD, 128], BF, name="tp", tag="tp")
                            nc.tensor.transpose(ptr[:], gx[:], ident[:])
                            nc.vector.tensor_copy(
                                paT[h * 32:h * 32 + 32,
                                    mi * 128:(mi + 1) * 128], ptr[:])

                    ag2_in = dram.tile([128, M], BF, name="ag2i", tag="ag2i")
                    ag2_out = dram.tile([D, M], BF, name="ag2o", tag="ag2o")
                    nc.sync.dma_start(ag2_in[:], paT[:])
                    nc.gpsimd.collective_compute(
                        "AllGather", OP.bypass,
                        replica_groups=PAIRS,
                        ins=[ag2_in.opt()], outs=[ag2_out.opt()])
                    paF = [p2e.tile([128, M], BF, name=f"paF{t}", tag=f"paF{t}")
                           for t in range(2)]
                    for t in range(2):
                        nc.sync.dma_start(
                            paF[t][:], ag2_out[t * 128:(t + 1) * 128, :])

                    w1_sb = [[lw(p2, SH_W1 + i * 128, SH_W1 + (i + 1) * 128,
                                 blk * 256 + o * 128, blk * 256 + (o + 1) * 128)
                              for o in range(2)] for i in range(2)]
                    b1_sb = [blobS_sb[:, SC_B1 + blk * 2 + t:
                                      SC_B1 + blk * 2 + t + 1]
                             for t in range(2)]
                    h1 = [p2e.tile([128, M], BF, name=f"h1{t}", tag=f"h1{t}")
                          for t in range(2)]
                    for o in range(2):
                        for mh in range(2):
                            ph = psum(128, 512)
                            for i in range(2):
                                nc.tensor.matmul(
                                    ph[:], w1_sb[i][o][:],
                                    paF[i][:, mh * 512:(mh + 1) * 512],
                                    start=(i == 0), stop=(i == 1))
                            nc.scalar.activation(
                                h1[o][:, mh * 512:(mh + 1) * 512], ph[:],
                                AF.Gelu, bias=b1_sb[o])

                    w2_sb = [[lw(p2, SH_W2 + i * 128, SH_W2 + (i + 1) * 128,
                                 blk * 256 + o * 128, blk * 256 + (o + 1) * 128)
                              for o in range(2)] for i in range(2)]
                    wr_sb = [[lw(p2, SH_WR + i * 128, SH_WR + (i + 1) * 128,
                                 blk * 256 + o * 128, blk * 256 + (o + 1) * 128)
                              for o in range(2)] for i in range(2)]
                    bc_sb = [blobS_sb[:, SC_BC + blk * 2 + t:
                                      SC_BC + blk * 2 + t + 1]
                             for t in range(2)]
                    xn = [p2e.tile([128, M], BF, name=f"xn{t}", tag=f"xn{t}")
                          for t in range(2)]
                    for o in range(2):
                        for mh in range(2):
                            po = psum(128, 512)
                            nc.tensor.matmul(
                                po[:], w2_sb[0][o][:],
                                h1[0][:, mh * 512:(mh + 1) * 512],
                                start=True, stop=False)
                            nc.tensor.matmul(
                                po[:], w2_sb[1][o][:],
                                h1[1][:, mh * 512:(mh + 1) * 512],
                                start=False, stop=False)
                            nc.tensor.matmul(
                                po[:], wr_sb[0][o][:],
                                xT[0][:, mh * 512:(mh + 1) * 512],
                                start=False, stop=False)
                            nc.tensor.matmul(
                                po[:], wr_sb[1][o][:],
                                xT[1][:, mh * 512:(mh + 1) * 512],
                                start=False, stop=True)
                            nc.scalar.activation(
                                xn[o][:, mh * 512:(mh + 1) * 512], po[:],
                                AF.Gelu, bias=bc_sb[o])
                    xT = xn

                for t in range(2):
                    nc.sync.dma_start(ag3_in[t * 128:(t + 1) * 128, :],
                                      xT[t][:])
                nc.gpsimd.collective_compute(
                    "AllGather", OP.bypass, replica_groups=ALL8,
                    ins=[ag3_in.opt()], outs=[ag3_out.opt()])

            # ---------------- P4: up (head pid over full N) ----------------
            with tc.tile_pool(name="p4", bufs=3) as p4, \
                 tc.tile_pool(name="p4keep", bufs=1) as p4k:
                vu = [p4k.tile([128, 4 * 33], BF, name=f"vu{i}", tag=f"vu{i}")
                      for i in range(8)]
                for b in range(B):
                    xb = [p4.tile([128, M], BF, name=f"xb{t}", tag=f"xb{t}")
                          for t in range(2)]
                    for t in range(2):
                        nc.sync.dma_start(
                            xb[t][:],
                            ag3_out[2 * b * D + t * 128:
                                    2 * b * D + (t + 1) * 128, :])
                    for mi in range(8):
                        pv = psum(128, KD)
                        for t in range(2):
                            nc.tensor.matmul(
                                pv[:], xb[t][:, mi * 128:(mi + 1) * 128],
                                wup_sb[t][:], start=(t == 0), stop=(t == 1))
                        nc.vector.tensor_copy(
                            vu[mi][:, b * 33:b * 33 + KD], pv[:])
                        if b == 0:
                            for bb in range(B):
                                nc.vector.memset(
                                    vu[mi][:, bb * 33 + 32:bb * 33 + 33], 1.0)

                # thrU (128, N) from the gathered t_up rows (matmul broadcast)
                tu_row = p4k.tile([1, N], BF, name="tur", tag="tur")
                for r in range(4):
                    nc.sync.dma_start(
                        tu_row[:, r * 1024:(r + 1) * 1024],
                        blob_out[R_TUP + r:R_TUP + r + 1, :])
                thrU = p4k.tile([128, N], BF, name="thrU", tag="thrU")
                for hf in range(8):
                    pb = pp.tile([128, 512], F32, name="pp", tag="pp")
                    nc.tensor.matmul(pb[:], ones_sb[:],
                                     tu_row[:, hf * 512:(hf + 1) * 512])
                    nc.vector.tensor_copy(thrU[:, hf * 512:(hf + 1) * 512],
                                          pb[:])

                # eu[ki] = exp(-c_h * mcT) * (mcT <= thrU), mcT via DMA transpose
                eu = [p4k.tile([128, N], BF, name=f"eu{i}", tag=f"eu{i}")
                      for i in range(8)]
                for ki in range(8):
                    mctT = p4.tile([128, N], BF, name="mctT", tag="mctT")
                    nc.sync.dma_start(mctT[:],
                                      mc_dram[:, ki * 128:(ki + 1) * 128],
                                      transpose=True)
                    msk = p4.tile([128, N], BF, name="umsk", tag="umsk")
                    nc.vector.tensor_tensor(msk[:], mctT[:], thrU[:], OP.is_le)
                    nc.scalar.activation(eu[ki][:], mctT[:], AF.Exp,
                                         scale=ncu_sb[:])
                    nc.vector.tensor_tensor(eu[ki][:], eu[ki][:], msk[:],
                                            OP.mult)

                deT = p4k.tile([KD, BN], BF, name="deT", tag="deT")
                for qi in range(32):
                    pd = psum(128, 4 * 33)
                    for ki in range(8):
                        nc.tensor.matmul(
                            pd[:], eu[ki][:, qi * 128:(qi + 1) * 128],
                            vu[ki][:], start=(ki == 0), stop=(ki == 7))
                    for b in range(B):
                        rc = small.tile([128, 1], F32, name="rc", tag="rc")
                        nc.vector.reciprocal(
                            rc[:], pd[:, b * 33 + 32:b * 33 + 33])
                        gx = small.tile([128, KD], BF, name="gx", tag="gx")
                        nc.scalar.activation(gx[:], pd[:, b * 33:b * 33 + KD],
                                             AF.Gelu, scale=rc[:])
                        ptr = ppt.tile([KD, 128], BF, name="tp", tag="tp")
                        nc.tensor.transpose(ptr[:], gx[:], ident[:])
                        nc.vector.tensor_copy(
                            deT[:, b * N + qi * 128:b * N + (qi + 1) * 128],
                            ptr[:])
                nc.sync.dma_start(ag4_in[:], deT[:])
                nc.gpsimd.collective_compute(
                    "AllGather", OP.bypass, replica_groups=ALL8,
                    ins=[ag4_in.opt()], outs=[ag4_out.opt()])

            # ---------------- P5: decoder on token shard ----------------
            TS = BN // NCORE  # 2048
            toff = pid * TS
            with tc.tile_pool(name="p5", bufs=2) as p5:
                dea = [p5.tile([128, TS], BF, name=f"dea{t}", tag=f"dea{t}")
                       for t in range(2)]
                for t in range(2):
                    nc.sync.dma_start(
                        dea[t][:],
                        ag4_out[t * 128:(t + 1) * 128, ds(toff, TS)])
                wd1 = [[lw(p5, SH_MISC + i * 128, SH_MISC + (i + 1) * 128,
                           o * 128, (o + 1) * 128) for o in range(2)]
                       for i in range(2)]
                bd1 = [blobS_sb[:, SC_BD1 + t:SC_BD1 + t + 1]
                       for t in range(2)]
                wd2 = [lw(p5, SH_MISC + 8, SH_MISC + 8 + 128,
                          770 + t, 771 + t) for t in range(2)]
                g = [p5.tile([128, TS], BF, name=f"g{t}", tag=f"g{t}")
                     for t in range(2)]
                for o in range(2):
                    for th in range(4):
                        pg = psum(128, 512)
                        for i in range(2):
                            nc.tensor.matmul(
                                pg[:], wd1[i][o][:],
                                dea[i][:, th * 512:(th + 1) * 512],
                                start=(i == 0), stop=(i == 1))
                        nc.scalar.activation(
                            g[o][:, th * 512:(th + 1) * 512], pg[:],
                            AF.Gelu, bias=bd1[o])
                osb = p5.tile([1, TS], F32, name="osb", tag="osb")
                for th in range(4):
                    p2o = psum(1, 512)
                    for i in range(2):
                        nc.tensor.matmul(
                            p2o[:], wd2[i][:],
                            g[i][:, th * 512:(th + 1) * 512],
                            start=(i == 0), stop=(i == 1))
                    nc.vector.tensor_copy(
                        osb[:, th * 512:(th + 1) * 512], p2o[:])
                nc.sync.dma_start(out_shard[:, :], osb[:])

    nc.compile()
    return nc


def _prep_inputs(inputs, m_cross, W_en, b_en, r_down, w_down, q_pa, k_pa,
                 v_pa, W1_mlp, b1_mlp, W2_mlp, b2_mlp, W_res, b_res, r_up,
                 w_up, W_de1, b_de1, W_de2, b_de2, y_mean, y_std):
    f32 = np.float32
    mc = np.asarray(m_cross, f32)
    mcb = mc.astype(NPBF)
    mcbf = mcb.astype(f32)

    # bf16-robust percentile thresholds: max passing bf16 value so the
    # device-side (bf16 mc <= thr) comparison reproduces the f32 mask
    kd_ = int(0.30 * (N - 1))          # 1228
    vkd = np.partition(mc, kd_, axis=0)[kd_, :]               # (M,)
    t_down_dev = np.where(mc <= vkd[None, :], mcbf, -np.inf).max(axis=0)
    ku_ = int(0.30 * (M - 1))          # 306
    vku = np.partition(mc, ku_, axis=1)[:, ku_]               # (N,)
    t_up_dev = np.where(mc <= vku[:, None], mcbf, -np.inf).max(axis=1)

    # encoder input (4, BN), group-major rows of 1024
    gx = np.linspace(0.0, 1.0, RES + 1, dtype=f32)[:-1]
    gxx = np.broadcast_to(gx[:, None], (RES, RES))
    gyy = np.broadcast_to(gx[None, :], (RES, RES))
    encf = np.zeros((4, BN), f32)
    encf[0, :] = np.tile(gxx.reshape(-1), B)
    encf[1, :] = np.tile(gyy.reshape(-1), B)
    encf[2, :] = np.asarray(inputs, f32).reshape(BN)
    enc_cat = encf.reshape(4, 16, 1024).transpose(1, 0, 2).reshape(64, 1024)

    c_down = np.tan(0.25 * np.pi * (1.0 + np.sin(np.asarray(r_down, f32)
                                                 .reshape(H)))).astype(f32)
    c_up = np.tan(0.25 * np.pi * (1.0 + np.sin(np.asarray(r_up, f32)
                                               .reshape(H)))).astype(f32)

    # ---- weight sheet (1792, 1024) ----
    sheet = np.zeros((SHEET_ROWS, 1024), NPBF)

    def cat_blocks(w):  # (NB, D, D) -> (D, NB*D)
        return np.asarray(w, f32).transpose(1, 0, 2).reshape(D, NB * D)

    def cat_heads(w):   # (NB, H, D, K) -> (D, NB*H*K)
        return np.asarray(w, f32).transpose(2, 0, 1, 3).reshape(D, NB * H * KD)

    sheet[SH_W1:SH_W1 + D] = cat_blocks(W1_mlp).astype(NPBF)
    sheet[SH_W2:SH_W2 + D] = cat_blocks(W2_mlp).astype(NPBF)
    sheet[SH_WR:SH_WR + D] = cat_blocks(W_res).astype(NPBF)
    sheet[SH_QP:SH_QP + D] = cat_heads(q_pa).astype(NPBF)
    sheet[SH_KP:SH_KP + D] = cat_heads(k_pa).astype(NPBF)
    sheet[SH_VP:SH_VP + D] = cat_heads(v_pa).astype(NPBF)
    sheet[SH_MISC:SH_MISC + D, 0:256] = np.asarray(W_de1, f32).astype(NPBF)
    sheet[SH_MISC:SH_MISC + D, 256:512] = (
        np.asarray(w_down, f32).transpose(1, 0, 2).reshape(D, H * KD)
        .astype(NPBF))
    sheet[SH_MISC:SH_MISC + D, 512:768] = (
        np.asarray(w_up, f32).transpose(1, 0, 2).reshape(D, H * KD)
        .astype(NPBF))
    wen4 = np.zeros((4, D), f32)
    wen4[:3, :] = np.asarray(W_en, f32)
    sheet[SH_MISC:SH_MISC + 4, 768:1024] = wen4.astype(NPBF)
    ystd = float(np.asarray(y_std, f32))
    ymean = float(np.asarray(y_mean, f32))
    wde2f = (np.asarray(W_de2, f32).reshape(D) * ystd).astype(NPBF)
    sheet[SH_MISC + 8:SH_MISC + 8 + 128, 770] = wde2f[0:128]
    sheet[SH_MISC + 8:SH_MISC + 8 + 128, 771] = wde2f[128:256]
    bde2f = float(np.asarray(b_de2, f32).reshape(-1)[0] * ystd + ymean)

    # ---- f32 constants (128, 36) ----
    bS = np.zeros((128, SCOLS), f32)
    bS[:, SC_NCD:SC_NCD + 8] = -c_down[None, :]
    bS[:, SC_NCU:SC_NCU + 8] = -c_up[None, :]
    ben = np.asarray(b_en, f32).reshape(D)
    bS[:, SC_BEN] = ben[0:128]
    bS[:, SC_BEN + 1] = ben[128:256]
    b1f = np.asarray(b1_mlp, f32).reshape(NB, D)
    bcf = (np.asarray(b2_mlp, f32) + np.asarray(b_res, f32)).reshape(NB, D)
    for blk in range(NB):
        for t in range(2):
            bS[:, SC_B1 + blk * 2 + t] = b1f[blk, t * 128:(t + 1) * 128]
            bS[:, SC_BC + blk * 2 + t] = bcf[blk, t * 128:(t + 1) * 128]
    bd1 = np.asarray(b_de1, f32).reshape(D)
    bS[:, SC_BD1] = bd1[0:128]
    bS[:, SC_BD1 + 1] = bd1[128:256]

    in_maps = []
    for c in range(NCORE):
        blob = np.zeros((BLOB_ROWS, 1024), NPBF)
        blob[0:512] = mcb[c * 512:(c + 1) * 512]
        blob[R_ENC:R_ENC + 8] = enc_cat[c * 8:(c + 1) * 8].astype(NPBF)
        blob[R_TDOWN] = t_down_dev.astype(NPBF)
        blob[R_TUP:R_TUP + 4] = t_up_dev.reshape(4, 1024).astype(NPBF)
        in_maps.append({
            "blob": blob,
            "sheet": sheet[c * (SHEET_ROWS // NCORE):
                           (c + 1) * (SHEET_ROWS // NCORE)],
            "blobS": bS,
        })
    return in_maps, bde2f


def kernel(**inputs):
    if "nc" not in _cache:
        _cache["nc"] = _build()
    nc = _cache["nc"]
    in_maps, bde2f = _prep_inputs(**inputs)
    res = run_bass_kernel_spmd(nc, in_maps, core_ids=list(range(NCORE)))
    shards = [res.results[c]["out_shard"].reshape(-1) + np.float32(bde2f)
              for c in range(NCORE)]
    out = np.concatenate(shards).astype(np.float32)
    return out.reshape(B, RES, RES, 1)


# revision 10
# speedup vs baseline: 9.7034x; 2.3152x over previous
"""Trainium2 Bass kernel for nn_LiteTransformer (sparse_attention).

Sharding (8 cores):
  - position-attention (down & up): by head (core c owns head c)
  - self-attention blocks: core c owns batch c//2, heads c%2*4..+4
  - decoder: token-sharded (2048 tokens per core)

Host->device transfer is the wall-clock bottleneck (axon tunnel ~120MB/s,
~0.2s per tensor latency), so inputs are packed into THREE small tensors
per core (~1.5MB total) instead of replicating the 16MB masked distance
matrices everywhere:
  - blob  (525,1024) bf16: m_cross row-shard + encoder-input slice +
    masked-percentile thresholds; AllGathered on device.
  - sheet (224,1024) bf16: 1/8 shard of all weights; AllGathered on device.
  - blobS (128,36)  f32 : per-head exp scales + biases (replicated).
The masked distance matrices exp(-c*(mc + big*(mc>thr))) are computed
on-device as exp(-c*mc) * (mc <= thr); thresholds are host-refined so the
bf16 comparison reproduces the exact f32 percentile mask.
"""

import numpy as np
import ml_dtypes

import jax
# run_bass_kernel_spmd builds a fresh jit closure per call; persist the XLA
# executable so repeat calls skip the ~0.5s re-compile (NEFF is already
# disk-cached separately).
jax.config.update("jax_compilation_cache_dir", "/tmp/jax_cache_kernel")
jax.config.update("jax_persistent_cache_min_entry_size_bytes", 0)
jax.config.update("jax_persistent_cache_min_compile_time_secs", 0)

import concourse.bass as bass
import concourse.mybir as mybir
import concourse.tile as tile
from concourse import bacc
from concourse.bass import ds
from concourse.bass_utils import run_bass_kernel_spmd
from concourse.masks import make_identity

BF = mybir.dt.bfloat16
F32 = mybir.dt.float32
AF = mybir.ActivationFunctionType
OP = mybir.AluOpType
NPBF = ml_dtypes.bfloat16

B, RES, N, M, H, D, KD, NB = 4, 64, 4096, 1024, 8, 256, 32, 4
BN = B * N
NCORE = 8
INV_SQRT_K = float(1.0 / np.sqrt(np.float32(KD)))
ALL8 = [list(range(NCORE))]
PAIRS = [[0, 1], [2, 3], [4, 5], [6, 7]]

# blob layout (per-core rows, width 1024 bf16)
BLOB_ROWS = 525          # 512 mc + 8 enc + 1 t_down + 4 t_up
R_ENC = 512
R_TDOWN = 520
R_TUP = 521
# sheet layout (global rows, width 1024 bf16)
SHEET_ROWS = 1792        # 224 per core
SH_W1, SH_W2, SH_WR = 0, 256, 512
SH_QP, SH_KP, SH_VP = 768, 1024, 1280
SH_MISC = 1536           # cols 0:256 wde1 | 256:512 wdown | 512:768 wup | 768: misc2
# misc2: rows SH_MISC..+4 cols 768:1024 = wen; wde2 halves at cols 770,771 rows +8..+136
# blobS cols
SC_NCD, SC_NCU, SC_BEN, SC_B1, SC_BC, SC_BD1, SCOLS = 0, 8, 16, 18, 26, 34, 36

_cache = {}


def _build():
    nc = bacc.Bacc("TRN2", target_bir_lowering=False, debug=False,
                   num_devices=NCORE)

    blob = nc.dram_tensor("blob", [BLOB_ROWS, 1024], BF,
                          kind="ExternalInput").ap()
    sheet = nc.dram_tensor("sheet", [SHEET_ROWS // NCORE, 1024], BF,
                           kind="ExternalInput").ap()
    blobS = nc.dram_tensor("blobS", [128, SCOLS], F32,
                           kind="ExternalInput").ap()
    out_shard = nc.dram_tensor("out_shard", [1, BN // NCORE], F32,
                               kind="ExternalOutput").ap()

    with tile.TileContext(nc) as tc:
        with (
            tc.tile_pool(name="dram", bufs=1, space="DRAM") as dram,
            tc.tile_pool(name="consts", bufs=1) as consts,
            tc.tile_pool(name="small", bufs=6) as small,
            tc.tile_pool(name="pp", bufs=4, space="PSUM") as pp,
            tc.tile_pool(name="pt", bufs=2, space="PSUM") as ppt,
        ):
            ident = consts.tile([128, 128], BF, name="ident", tag="ident")
            make_identity(nc, ident)
            pid = nc.sync.partition_id()

            # ---- gather the packed inputs across cores ----
            # (collectives cannot read IO tensors; bounce through DRAM scratch)
            blob_in = dram.tile([BLOB_ROWS, 1024], BF, name="blobi",
                                tag="blobi")
            nc.sync.dma_start(blob_in[:, :], blob[:, :])
            blob_out = dram.tile([NCORE * BLOB_ROWS, 1024], BF, name="blobo",
                                 tag="blobo", addr_space="Shared")
            nc.gpsimd.collective_compute(
                "AllGather", OP.bypass, replica_groups=ALL8,
                ins=[blob_in.opt()], outs=[blob_out.opt()])
            sheet_in = dram.tile([SHEET_ROWS // NCORE, 1024], BF,
                                 name="sheeti", tag="sheeti")
            nc.sync.dma_start(sheet_in[:, :], sheet[:, :])
            sheet_out = dram.tile([SHEET_ROWS, 1024], BF, name="sheeto",
                                  tag="sheeto", addr_space="Shared")
            nc.gpsimd.collective_compute(
                "AllGather", OP.bypass, replica_groups=ALL8,
                ins=[sheet_in.opt()], outs=[sheet_out.opt()])

            # clean (4096, 1024) m_cross scratch out of the gathered chunks
            mc_dram = dram.tile([N, 1024], BF, name="mcd", tag="mcd")
            for c in range(NCORE):
                nc.sync.dma_start(
                    mc_dram[c * 512:(c + 1) * 512, :],
                    blob_out[c * BLOB_ROWS:c * BLOB_ROWS + 512, :])

            blobS_sb = consts.tile([128, SCOLS], F32, name="bS", tag="bS")
            nc.sync.dma_start(blobS_sb[:], blobS[:, :])
            ncd_sb = consts.tile([128, 1], F32, name="ncd", tag="ncd")
            nc.sync.dma_start(ncd_sb[:], blobS[0:128, ds(SC_NCD + pid, 1)])
            ncu_sb = consts.tile([128, 1], F32, name="ncu", tag="ncu")
            nc.sync.dma_start(ncu_sb[:], blobS[0:128, ds(SC_NCU + pid, 1)])

            # thresholds broadcast to 128 partitions (ones ⊗ row via matmul)
            ones_sb = consts.tile([1, 128], BF, name="ones", tag="ones")
            nc.vector.memset(ones_sb[:], 1.0)
            td_row = consts.tile([1, 1024], BF, name="tdr", tag="tdr")
            nc.sync.dma_start(td_row[:], blob_out[R_TDOWN:R_TDOWN + 1, :])
            thrD = consts.tile([128, 1024], BF, name="thrD", tag="thrD")
            for hf in range(2):
                pb = pp.tile([128, 512], F32, name="pp", tag="pp")
                nc.tensor.matmul(pb[:], ones_sb[:],
                                 td_row[:, hf * 512:(hf + 1) * 512])
                nc.vector.tensor_copy(thrD[:, hf * 512:(hf + 1) * 512], pb[:])

            wen_sb = consts.tile([4, 256], BF, name="wen", tag="wen")
            nc.sync.dma_start(wen_sb[:], sheet_out[SH_MISC:SH_MISC + 4, 768:1024])
            wdn_sb = []
            wup_sb = []
            for t in range(2):
                w = consts.tile([128, KD], BF, name=f"wdn{t}", tag=f"wdn{t}")
                nc.sync.dma_start(
                    w[:], sheet_out[SH_MISC + t * 128:SH_MISC + (t + 1) * 128,
                                    ds(256 + pid * KD, KD)])
                wdn_sb.append(w)
                w = consts.tile([128, KD], BF, name=f"wupt{t}", tag=f"wupt{t}")
                nc.sync.dma_start(
                    w[:], sheet_out[SH_MISC + t * 128:SH_MISC + (t + 1) * 128,
                                    ds(512 + pid * KD, KD)])
                wup_sb.append(w)
            ben_sb = [blobS_sb[:, SC_BEN + t:SC_BEN + t + 1] for t in range(2)]

            ag1_in = dram.tile([128, M], BF, name="ag1i", tag="ag1i")
            ag1_out = dram.tile([NCORE * 128, M], BF, name="ag1o", tag="ag1o",
                                addr_space="Shared")
            ag3_in = dram.tile([D, M], BF, name="ag3i", tag="ag3i")
            ag3_out = dram.tile([NCORE * D, M], BF, name="ag3o", tag="ag3o",
                                addr_space="Shared")
            ag4_in = dram.tile([KD, BN], BF, name="ag4i", tag="ag4i")
            ag4_out = dram.tile([NCORE * KD, BN], BF, name="ag4o", tag="ag4o",
                                addr_space="Shared")

            def psum(p, f, dt=F32):
                return pp.tile([p, f], dt, name="pp", tag="pp")

            _lwn = [0]

            def lw(pool, p0, p1, f0, f1, dt=BF):
                # load a (p1-p0, f1-f0) tile from sheet_out
                _lwn[0] += 1
                t = pool.tile([p1 - p0, f1 - f0], dt, name=f"lw{_lwn[0]}",
                              tag=f"lw{_lwn[0]}")
                nc.sync.dma_start(t[:], sheet_out[p0:p1, f0:f1])
                return t

            # ---------------- P1: down (head pid over full N) ----------------
            with tc.tile_pool(name="p1", bufs=3) as p1, \
                 tc.tile_pool(name="p1keep", bufs=1) as p1k:
                # encoder input (4, BN) from the 16 gathered groups
                enc_sb = p1k.tile([4, BN], BF, name="enc_sb", tag="enc_sb")
                for g in range(16):
                    src = (g // 2) * BLOB_ROWS + R_ENC + (g % 2) * 4
                    nc.sync.dma_start(enc_sb[:, g * 1024:(g + 1) * 1024],
                                      blob_out[src:src + 4, :])
                v_all = [p1k.tile([128, 4 * 33], BF, name=f"va{i}", tag=f"va{i}")
                         for i in range(32)]
                for b in range(B):
                    for ni in range(32):
                        off = b * N + ni * 128
                        enT = []
                        for t in range(2):
                            pe = psum(128, 128)
                            nc.tensor.matmul(
                                pe[:], wen_sb[:, t * 128:(t + 1) * 128],
                                enc_sb[:, off:off + 128])
                            g = p1.tile([128, 128], BF, name="enT", tag="enT")
                            nc.scalar.activation(g[:], pe[:], AF.Gelu,
                                                 bias=ben_sb[t])
                            enT.append(g)
                        pv = psum(128, KD)
                        for t in range(2):
                            nc.tensor.matmul(pv[:], enT[t][:], wdn_sb[t][:],
                                             start=(t == 0), stop=(t == 1))
                        nc.vector.tensor_copy(
                            v_all[ni][:, b * 33:b * 33 + KD], pv[:])
                        if b == 0:
                            for bb in range(B):
                                nc.vector.memset(
                                    v_all[ni][:, bb * 33 + 32:bb * 33 + 33],
                                    1.0)

                # a_sb[ni] = exp(-c_h * mc) * (mc <= thr)
                a_sb = [p1k.tile([128, M], BF, name=f"as{i}", tag=f"as{i}")
                        for i in range(32)]
                for ni in range(32):
                    mct = p1.tile([128, M], BF, name="mct", tag="mct")
                    nc.sync.dma_start(mct[:],
                                      mc_dram[ni * 128:(ni + 1) * 128, :])
                    msk = p1.tile([128, M], BF, name="msk", tag="msk")
                    nc.vector.tensor_tensor(msk[:], mct[:], thrD[:], OP.is_le)
                    nc.scalar.activation(a_sb[ni][:], mct[:], AF.Exp,
                                         scale=ncd_sb[:])
                    nc.vector.tensor_tensor(a_sb[ni][:], a_sb[ni][:], msk[:],
                                            OP.mult)

                xhT = p1k.tile([128, M], BF, name="xhT", tag="xhT")
                for mi in range(8):
                    px = psum(128, 4 * 33)
                    for ni in range(32):
                        nc.tensor.matmul(
                            px[:], a_sb[ni][:, mi * 128:(mi + 1) * 128],
                            v_all[ni][:], start=(ni == 0), stop=(ni == 31))
                    for b in range(B):
                        rc = small.tile([128, 1], F32, name="rc", tag="rc")
                        nc.vector.reciprocal(
                            rc[:], px[:, b * 33 + 32:b * 33 + 33])
                        gx = small.tile([128, KD], BF, name="gx", tag="gx")
                        nc.scalar.activation(gx[:], px[:, b * 33:b * 33 + KD],
                                             AF.Gelu, scale=rc[:])
                        ptr = ppt.tile([KD, 128], BF, name="tp", tag="tp")
                        nc.tensor.transpose(ptr[:], gx[:], ident[:])
                        nc.vector.tensor_copy(
                            xhT[b * 32:b * 32 + 32,
                                mi * 128:(mi + 1) * 128], ptr[:])
                nc.sync.dma_start(ag1_in[:], xhT[:])
                nc.gpsimd.collective_compute(
                    "AllGather", OP.bypass, replica_groups=ALL8,
                    ins=[ag1_in.opt()], outs=[ag1_out.opt()])

            # ---------------- P2: blocks ----------------
            b0x32 = (pid // 2) * 32
            # static range clamp only — the emitted runtime-assert sequencer
            # instruction faults the exec unit on this runtime, so skip it
            hcol = nc.s_assert_within((pid - (pid // 2) * 2) * 128, 0, 128,
                                      skip_runtime_assert=True)
            with tc.tile_pool(name="p2", bufs=2) as p2, \
                 tc.tile_pool(name="p2e", bufs=1) as p2e:
                xT = [p2e.tile([128, M], BF, name=f"xT{t}", tag=f"xT{t}")
                      for t in range(2)]
                for hh in range(H):
                    nc.sync.dma_start(
                        xT[hh // 4][(hh % 4) * 32:(hh % 4) * 32 + 32, :],
                        ag1_out[ds(hh * 128 + b0x32, 32), :])

                for blk in range(NB):
                    qp_sb, kp_sb, vp_sb = [], [], []
                    for t in range(2):
                        for dst, base in ((qp_sb, SH_QP), (kp_sb, SH_KP),
                                          (vp_sb, SH_VP)):
                            _lwn[0] += 1
                            w = p2.tile([128, 128], BF, name=f"lw{_lwn[0]}",
                                        tag=f"lw{_lwn[0]}")
                            nc.sync.dma_start(
                                w[:], sheet_out[base + t * 128:
                                                base + (t + 1) * 128,
                                                ds(blk * 256 + hcol, 128)])
                            dst.append(w)

                    qt = p2e.tile([128, M], BF, name="qt", tag="qt")
                    kt = p2e.tile([128, M], BF, name="kt", tag="kt")
                    for dst, wsb in ((qt, qp_sb), (kt, kp_sb)):
                        for mh in range(2):
                            pq = psum(128, 512)
                            for t in range(2):
                                nc.tensor.matmul(
                                    pq[:], wsb[t][:],
                                    xT[t][:, mh * 512:(mh + 1) * 512],
                                    start=(t == 0), stop=(t == 1))
                            nc.vector.tensor_copy(
                                dst[:, mh * 512:(mh + 1) * 512], pq[:])
                    qh = [p2e.tile([KD, M], BF, name=f"qh{h}", tag=f"qh{h}")
                          for h in range(4)]
                    kh = [p2e.tile([KD, M], BF, name=f"kh{h}", tag=f"kh{h}")
                          for h in range(4)]
                    for h in range(4):
                        nc.vector.tensor_copy(qh[h][:],
                                              qt[h * 32:h * 32 + 32, :])
                        nc.vector.tensor_copy(kh[h][:],
                                              kt[h * 32:h * 32 + 32, :])

                    vh = [p2e.tile([128, 4 * 33], BF, name=f"vh{ni}",
                                   tag=f"vh{ni}") for ni in range(8)]
                    for ni in range(8):
                        pvv = psum(128, 128)
                        for t in range(2):
                            nc.tensor.matmul(
                                pvv[:],
                                xT[t][:, ni * 128:(ni + 1) * 128],
                                vp_sb[t][:], start=(t == 0), stop=(t == 1))
                        for h in range(4):
                            nc.vector.tensor_copy(
                                vh[ni][:, h * 33:h * 33 + KD],
                                pvv[:, h * 32:h * 32 + 32])
                            nc.vector.memset(
                                vh[ni][:, h * 33 + 32:h * 33 + 33], 1.0)

                    es = [[p2e.tile([128, M], BF, name=f"es{h}_{ni}",
                                    tag=f"es{h}_{ni}")
                           for ni in range(8)] for h in range(4)]
                    for h in range(4):
                        for ni in range(8):
                            for mh in range(2):
                                psc = psum(128, 512)
                                nc.tensor.matmul(
                                    psc[:],
                                    kh[h][:, ni * 128:(ni + 1) * 128],
                                    qh[h][:, mh * 512:(mh + 1) * 512])
                                nc.scalar.activation(
                                    es[h][ni][:, mh * 512:(mh + 1) * 512],
                                    psc[:], AF.Exp, scale=INV_SQRT_K)

                    paT = p2e.tile([128, M], BF, name="paT", tag="paT")
                    for mi in range(8):
                        for h in range(4):
                            pa = psum(128, 33)
                            for ni in range(8):
                                nc.tensor.matmul(
                                    pa[:],
                                    es[h][ni][:, mi * 128:(mi + 1) * 128],
                                    vh[ni][:, h * 33:h * 33 + 33],
                                    start=(ni == 0), stop=(ni == 7))
                            rc = small.tile([128, 1], F32, name="rc", tag="rc")
                            nc.vector.reciprocal(rc[:], pa[:, 32:33])
                            gx = small.tile([128, KD], BF, name="gx", tag="gx")
                            nc.scalar.activation(gx[:], pa[:, 0:KD], AF.Gelu,
                                                 scale=rc[:])
                            ptr = ppt.tile([KD, 128], BF, name="tp", tag="tp")
                            nc.tensor.transpose(ptr[:], gx[:], ident[:])
                            nc.vector.tensor_copy(
                                paT[h * 32:h * 32 + 32,
                                    mi * 128:(mi + 1) * 128], ptr[:])

                    ag2_in = dram.tile([128, M], BF, name="ag2i", tag="ag2i")
                    ag2_out = dram.tile([D, M], BF, name="ag2o", tag="ag2o")
                    nc.sync.dma_start(ag2_in[:], paT[:])
                    nc.gpsimd.collective_compute(
                        "AllGather", OP.bypass,
                        replica_groups=PAIRS,
                        ins=[ag2_in.opt()], outs=[ag2_out.opt()])
                    paF = [p2e.tile([128, M], BF, name=f"paF{t}", tag=f"paF{t}")
                           for t in range(2)]
                    for t in range(2):
                        nc.sync.dma_start(
                            paF[t][:], ag2_out[t * 128:(t + 1) * 128, :])

                    w1_sb = [[lw(p2, SH_W1 + i * 128, SH_W1 + (i + 1) * 128,
                                 blk * 256 + o * 128, blk * 256 + (o + 1) * 128)
                              for o in range(2)] for i in range(2)]
                    b1_sb = [blobS_sb[:, SC_B1 + blk * 2 + t:
                                      SC_B1 + blk * 2 + t + 1]
                             for t in range(2)]
                    h1 = [p2e.tile([128, M], BF, name=f"h1{t}", tag=f"h1{t}")
                          for t in range(2)]
                    for o in range(2):
                        for mh in range(2):
                            ph = psum(128, 512)
                            for i in range(2):
                                nc.tensor.matmul(
                                    ph[:], w1_sb[i][o][:],
                                    paF[i][:, mh * 512:(mh + 1) * 512],
                                    start=(i == 0), stop=(i == 1))
                            nc.scalar.activation(
                                h1[o][:, mh * 512:(mh + 1) * 512], ph[:],
                                AF.Gelu, bias=b1_sb[o])

                    w2_sb = [[lw(p2, SH_W2 + i * 128, SH_W2 + (i + 1) * 128,
                                 blk * 256 + o * 128, blk * 256 + (o + 1) * 128)
                              for o in range(2)] for i in range(2)]
                    wr_sb = [[lw(p2, SH_WR + i * 128, SH_WR + (i + 1) * 128,
                                 blk * 256 + o * 128, blk * 256 + (o + 1) * 128)
                              for o in range(2)] for i in range(2)]
                    bc_sb = [blobS_sb[:, SC_BC + blk * 2 + t:
                                      SC_BC + blk * 2 + t + 1]
                             for t in range(2)]
                    xn = [p2e.tile([128, M], BF, name=f"xn{t}", tag=f"xn{t}")
                          for t in range(2)]
                    for o in range(2):
                        for mh in range(2):
                            po = psum(128, 512)
                            nc.tensor.matmul(
                                po[:], w2_sb[0][o][:],
                                h1[0][:, mh * 512:(mh + 1) * 512],
                                start=True, stop=False)
                            nc.tensor.matmul(
                                po[:], w2_sb[1][o][:],
                                h1[1][:, mh * 512:(mh + 1) * 512],
                                start=False, stop=False)
                            nc.tensor.matmul(
                                po[:], wr_sb[0][o][:],
                                xT[0][:, mh * 512:(mh + 1) * 512],
                                start=False, stop=False)
                            nc.tensor.matmul(
                                po[:], wr_sb[1][o][:],
                                xT[1][:, mh * 512:(mh + 1) * 512],
                                start=False, stop=True)
                            nc.scalar.activation(
                                xn[o][:, mh * 512:(mh + 1) * 512], po[:],
                                AF.Gelu, bias=bc_sb[o])
                    xT = xn

                for t in range(2):
                    nc.sync.dma_start(ag3_in[t * 128:(t + 1) * 128, :],
                                      xT[t][:])
                nc.gpsimd.collective_compute(
                    "AllGather", OP.bypass, replica_groups=ALL8,
                    ins=[ag3_in.opt()], outs=[ag3_out.opt()])

            # ---------------- P4: up (head pid over full N) ----------------
            with tc.tile_pool(name="p4", bufs=3) as p4, \
                 tc.tile_pool(name="p4keep", bufs=1) as p4k:
                vu = [p4k.tile([128, 4 * 33], BF, name=f"vu{i}", tag=f"vu{i}")
                      for i in range(8)]
                for b in range(B):
                    xb = [p4.tile([128, M], BF, name=f"xb{t}", tag=f"xb{t}")
                          for t in range(2)]
                    for t in range(2):
                        nc.sync.dma_start(
                            xb[t][:],
                            ag3_out[2 * b * D + t * 128:
                                    2 * b * D + (t + 1) * 128, :])
                    for mi in range(8):
                        pv = psum(128, KD)
                        for t in range(2):
                            nc.tensor.matmul(
                                pv[:], xb[t][:, mi * 128:(mi + 1) * 128],
                                wup_sb[t][:], start=(t == 0), stop=(t == 1))
                        nc.vector.tensor_copy(
                            vu[mi][:, b * 33:b * 33 + KD], pv[:])
                        if b == 0:
                            for bb in range(B):
                                nc.vector.memset(
                                    vu[mi][:, bb * 33 + 32:bb * 33 + 33], 1.0)

                # thrU (128, N) from the gathered t_up rows (matmul broadcast)
                tu_row = p4k.tile([1, N], BF, name="tur", tag="tur")
                for r in range(4):
                    nc.sync.dma_start(
                        tu_row[:, r * 1024:(r + 1) * 1024],
                        blob_out[R_TUP + r:R_TUP + r + 1, :])
                thrU = p4k.tile([128, N], BF, name="thrU", tag="thrU")
                for hf in range(8):
                    pb = pp.tile([128, 512], F32, name="pp", tag="pp")
                    nc.tensor.matmul(pb[:], ones_sb[:],
                                     tu_row[:, hf * 512:(hf + 1) * 512])
                    nc.vector.tensor_copy(thrU[:, hf * 512:(hf + 1) * 512],
                                          pb[:])

                # eu[ki] = exp(-c_h * mcT) * (mcT <= thrU), mcT via DMA transpose
                eu = [p4k.tile([128, N], BF, name=f"eu{i}", tag=f"eu{i}")
                      for i in range(8)]
                for ki in range(8):
                    mctT = p4.tile([128, N], BF, name="mctT", tag="mctT")
                    nc.sync.dma_start(mctT[:],
                                      mc_dram[:, ki * 128:(ki + 1) * 128],
                                      transpose=True)
                    msk = p4.tile([128, N], BF, name="umsk", tag="umsk")
                    nc.vector.tensor_tensor(msk[:], mctT[:], thrU[:], OP.is_le)
                    nc.scalar.activation(eu[ki][:], mctT[:], AF.Exp,
                                         scale=ncu_sb[:])
                    nc.vector.tensor_tensor(eu[ki][:], eu[ki][:], msk[:],
                                            OP.mult)

                deT = p4k.tile([KD, BN], BF, name="deT", tag="deT")
                for qi in range(32):
                    pd = psum(128, 4 * 33)
                    for ki in range(8):
                        nc.tensor.matmul(
                            pd[:], eu[ki][:, qi * 128:(qi + 1) * 128],
                            vu[ki][:], start=(ki == 0), stop=(ki == 7))
                    for b in range(B):
                        rc = small.tile([128, 1], F32, name="rc", tag="rc")
                        nc.vector.reciprocal(
                            rc[:], pd[:, b * 33 + 32:b * 33 + 33])
                        gx = small.tile([128, KD], BF, name="gx", tag="gx")
                        nc.scalar.activation(gx[:], pd[:, b * 33:b * 33 + KD],
                                             AF.Gelu, scale=rc[:])
                        ptr = ppt.tile([KD, 128], BF, name="tp", tag="tp")
                        nc.tensor.transpose(ptr[:], gx[:], ident[:])
                        nc.vector.tensor_copy(
                            deT[:, b * N + qi * 128:b * N + (qi + 1) * 128],
                            ptr[:])
                nc.sync.dma_start(ag4_in[:], deT[:])
                nc.gpsimd.collective_compute(
                    "AllGather", OP.bypass, replica_groups=ALL8,
                    ins=[ag4_in.opt()], outs=[ag4_out.opt()])

            # ---------------- P5: decoder on token shard ----------------
            TS = BN // NCORE  # 2048
            toff = pid * TS
            with tc.tile_pool(name="p5", bufs=2) as p5:
                dea = [p5.tile([128, TS], BF, name=f"dea{t}", tag=f"dea{t}")
                       for t in range(2)]
                for t in range(2):
                    nc.sync.dma_start(
                        dea[t][:],
                        ag4_out[t * 128:(t + 1) * 128, ds(toff, TS)])
                wd1 = [[lw(p5, SH_MISC + i * 128, SH_MISC + (i + 1) * 128,
                           o * 128, (o + 1) * 128) for o in range(2)]
                       for i in range(2)]
                bd1 = [blobS_sb[:, SC_BD1 + t:SC_BD1 + t + 1]
                       for t in range(2)]
                wd2 = [lw(p5, SH_MISC + 8, SH_MISC + 8 + 128,
                          770 + t, 771 + t) for t in range(2)]
                g = [p5.tile([128, TS], BF, name=f"g{t}", tag=f"g{t}")
                     for t in range(2)]
                for o in range(2):
                    for th in range(4):
                        pg = psum(128, 512)
                        for i in range(2):
                            nc.tensor.matmul(
                                pg[:], wd1[i][o][:],
                                dea[i][:, th * 512:(th + 1) * 512],
                                start=(i == 0), stop=(i == 1))
                        nc.scalar.activation(
                            g[o][:, th * 512:(th + 1) * 512], pg[:],
                            AF.Gelu, bias=bd1[o])
                osb = p5.tile([1, TS], F32, name="osb", tag="osb")
                for th in range(4):
                    p2o = psum(1, 512)
                    for i in range(2):
                        nc.tensor.matmul(
                            p2o[:], wd2[i][:],
                            g[i][:, th * 512:(th + 1) * 512],
                            start=(i == 0), stop=(i == 1))
                    nc.vector.tensor_copy(
                        osb[:, th * 512:(th + 1) * 512], p2o[:])
                nc.sync.dma_start(out_shard[:, :], osb[:])

    nc.compile()
    return nc


def _prep_inputs(inputs, m_cross, W_en, b_en, r_down, w_down, q_pa, k_pa,
                 v_pa, W1_mlp, b1_mlp, W2_mlp, b2_mlp, W_res, b_res, r_up,
                 w_up, W_de1, b_de1, W_de2, b_de2, y_mean, y_std):
    f32 = np.float32
    mc = np.asarray(m_cross, f32)
    mcb = mc.astype(NPBF)
    mcbf = mcb.astype(f32)

    # bf16-robust percentile thresholds: max passing bf16 value so the
    # device-side (bf16 mc <= thr) comparison reproduces the f32 mask
    kd_ = int(0.30 * (N - 1))          # 1228
    vkd = np.partition(mc, kd_, axis=0)[kd_, :]               # (M,)
    t_down_dev = np.where(mc <= vkd[None, :], mcbf, -np.inf).max(axis=0)
    ku_ = int(0.30 * (M - 1))          # 306
    vku = np.partition(mc, ku_, axis=1)[:, ku_]               # (N,)
    t_up_dev = np.where(mc <= vku[:, None], mcbf, -np.inf).max(axis=1)

    # encoder input (4, BN), group-major rows of 1024
    gx = np.linspace(0.0, 1.0, RES + 1, dtype=f32)[:-1]
    gxx = np.broadcast_to(gx[:, None], (RES, RES))
    gyy = np.broadcast_to(gx[None, :], (RES, RES))
    encf = np.zeros((4, BN), f32)
    encf[0, :] = np.tile(gxx.reshape(-1), B)
    encf[1, :] = np.tile(gyy.reshape(-1), B)
    encf[2, :] = np.asarray(inputs, f32).reshape(BN)
    enc_cat = encf.reshape(4, 16, 1024).transpose(1, 0, 2).reshape(64, 1024)

    c_down = np.tan(0.25 * np.pi * (1.0 + np.sin(np.asarray(r_down, f32)
                                                 .reshape(H)))).astype(f32)
    c_up = np.tan(0.25 * np.pi * (1.0 + np.sin(np.asarray(r_up, f32)
                                               .reshape(H)))).astype(f32)

    # ---- weight sheet (1792, 1024) ----
    sheet = np.zeros((SHEET_ROWS, 1024), NPBF)

    def cat_blocks(w):  # (NB, D, D) -> (D, NB*D)
        return np.asarray(w, f32).transpose(1, 0, 2).reshape(D, NB * D)

    def cat_heads(w):   # (NB, H, D, K) -> (D, NB*H*K)
        return np.asarray(w, f32).transpose(2, 0, 1, 3).reshape(D, NB * H * KD)

    sheet[SH_W1:SH_W1 + D] = cat_blocks(W1_mlp).astype(NPBF)
    sheet[SH_W2:SH_W2 + D] = cat_blocks(W2_mlp).astype(NPBF)
    sheet[SH_WR:SH_WR + D] = cat_blocks(W_res).astype(NPBF)
    sheet[SH_QP:SH_QP + D] = cat_heads(q_pa).astype(NPBF)
    sheet[SH_KP:SH_KP + D] = cat_heads(k_pa).astype(NPBF)
    sheet[SH_VP:SH_VP + D] = cat_heads(v_pa).astype(NPBF)
    sheet[SH_MISC:SH_MISC + D, 0:256] = np.asarray(W_de1, f32).astype(NPBF)
    sheet[SH_MISC:SH_MISC + D, 256:512] = (
        np.asarray(w_down, f32).transpose(1, 0, 2).reshape(D, H * KD)
        .astype(NPBF))
    sheet[SH_MISC:SH_MISC + D, 512:768] = (
        np.asarray(w_up, f32).transpose(1, 0, 2).reshape(D, H * KD)
        .astype(NPBF))
    wen4 = np.zeros((4, D), f32)
    wen4[:3, :] = np.asarray(W_en, f32)
    sheet[SH_MISC:SH_MISC + 4, 768:1024] = wen4.astype(NPBF)
    ystd = float(np.asarray(y_std, f32))
    ymean = float(np.asarray(y_mean, f32))
    wde2f = (np.asarray(W_de2, f32).reshape(D) * ystd).astype(NPBF)
    sheet[SH_MISC + 8:SH_MISC + 8 + 128, 770] = wde2f[0:128]
    sheet[SH_MISC + 8:SH_MISC + 8 + 128, 771] = wde2f[128:256]
    bde2f = float(np.asarray(b_de2, f32).reshape(-1)[0] * ystd + ymean)

    # ---- f32 constants (128, 36) ----
    bS = np.zeros((128, SCOLS), f32)
    bS[:, SC_NCD:SC_NCD + 8] = -c_down[None, :]
    bS[:, SC_NCU:SC_NCU + 8] = -c_up[None, :]
    ben = np.asarray(b_en, f32).reshape(D)
    bS[:, SC_BEN] = ben[0:128]
    bS[:, SC_BEN + 1] = ben[128:256]
    b1f = np.asarray(b1_mlp, f32).reshape(NB, D)
    bcf = (np.asarray(b2_mlp, f32) + np.asarray(b_res, f32)).reshape(NB, D)
    for blk in range(NB):
        for t in range(2):
            bS[:, SC_B1 + blk * 2 + t] = b1f[blk, t * 128:(t + 1) * 128]
            bS[:, SC_BC + blk * 2 + t] = bcf[blk, t * 128:(t + 1) * 128]
    bd1 = np.asarray(b_de1, f32).reshape(D)
    bS[:, SC_BD1] = bd1[0:128]
    bS[:, SC_BD1 + 1] = bd1[128:256]

    in_maps = []
    for c in range(NCORE):
        blob = np.zeros((BLOB_ROWS, 1024), NPBF)
        blob[0:512] = mcb[c * 512:(c + 1) * 512]
        blob[R_ENC:R_ENC + 8] = enc_cat[c * 8:(c + 1) * 8].astype(NPBF)
        blob[R_TDOWN] = t_down_dev.astype(NPBF)
        blob[R_TUP:R_TUP + 4] = t_up_dev.reshape(4, 1024).astype(NPBF)
        in_maps.append({
            "blob": blob,
            "sheet": sheet[c * (SHEET_ROWS // NCORE):
                           (c + 1) * (SHEET_ROWS // NCORE)],
            "blobS": bS,
        })
    return in_maps, bde2f


def kernel(**inputs):
    if "nc" not in _cache:
        _cache["nc"] = _build()
    nc = _cache["nc"]
    in_maps, bde2f = _prep_inputs(**inputs)
    res = run_bass_kernel_spmd(nc, in_maps, core_ids=list(range(NCORE)))
    shards = [res.results[c]["out_shard"].reshape(-1) + np.float32(bde2f)
              for c in range(NCORE)]
    out = np.concatenate(shards).astype(np.float32)
    return out.reshape(B, RES, RES, 1)
